# revision 16
# baseline (speedup 1.0000x reference)
"""Trainium2 Bass kernel for nn_DEC_26139170963600 (vq_codebook).

Reference computation:
  4x strided conv1d (stride 2, VALID) with LeakyReLU(0.1) between layers,
  flatten -> soft VQ assignment over 64 centers:
      d2 = ||z||^2 + ||c||^2 - 2 z.c
      q  = (1/(1+d2)) row-normalized            (alpha=1 -> exponent is 1)

Sharding: data-parallel over batch N=256 across 8 cores (32 samples/core).
Weights / centers replicated. No cross-device communication.

Per-core kernel design (all-fp8e4 conv path, ~2.3e-3 max rel err vs fp32):
  - Convs run as fp8 DoubleRow matmuls: each instruction contracts TWO taps
    (lhsT (128,2,128) pair of tap-weight matrices, rhs (128,2,G,Lout) pair of
    strided input slices) at 0.5 PE cycles per output column -- 4x the bf16
    rate per MAC.  Odd tap counts are padded with a zero tap (conv1 15->16,
    conv3 7->8); the pad tap's reads stay finite (slack columns memset to 0).
  - PSUM eviction is a single ACT op: Prelu(y + b) with alpha=0.1 (HW Prelu is
    exact; HW Lrelu is broken - returns 0.01x on negatives).  Outputs written
    directly as fp8 for the next layer.  conv4 evicts Identity+bias into z.
  - z is stored position-major (l*32 + n) so the distance matmul pairs two
    positions per DR instruction with 16-aligned pair strides:
      d2 cross term: lhsT = z-pair (128,2,32), rhs = cr-pair (128,2,64)
      ||z||^2:       same lhsT vs itself -> (32,32) Gram, diag extracted with
                     a (32,32) identity mult + row-reduce on DVE.
    1 + ||c||^2 comes in as a host-precomputed (32,64) fp32 tile (exact).
  - q = reciprocal(1+d2) row-normalized on DVE, DMA out as fp32.
  - PE pre-warm: dummy matmuls bridge the DMA lead-in so the cost of real conv
    work is billed at the full 2.4 GHz p-state (PE clock ramps over ~3us).

fp16 matmuls hard-fault trn2 here (NRT_EXEC_UNIT_UNRECOVERABLE) - do not use.
"""

import os
import sys

import numpy as np
import ml_dtypes

for _p in ("/opt/trn_rl_repo",):
    if _p not in sys.path and os.path.isdir(_p):
        sys.path.insert(0, _p)

import concourse.bacc as bacc  # noqa: E402
import concourse.mybir as mybir  # noqa: E402
import concourse.tile as tile  # noqa: E402
from concourse import bass_utils  # noqa: E402

FP8 = mybir.dt.float8e4
F32 = mybir.dt.float32
AF = mybir.ActivationFunctionType
OP = mybir.AluOpType
PM = mybir.MatmulPerfMode
NP8 = ml_dtypes.float8_e4m3  # maps to mybir float8e4 (max +-240)

N_CORES = 8
NS = 32          # samples per core
C = 128          # channels
KCENT = 64       # number of centers
LFIN = 59        # final length
LPAD = 60        # padded (even) final length for position-pair matmuls
NPAIR = LPAD // 2

# (K taps, L_in row stride, L_out, G samples per matmul, tap pairs)
CFG = [
    (15, 1024, 505, 1, 8),
    (12, 505, 247, 2, 6),
    (7, 248, 121, 4, 4),   # h2 rows padded 247->248 (pad-tap read slack)
    (4, 121, 59, 8, 2),
]
N_WARMUP = 24  # dummy PE matmuls bridging the DMA lead-in (p-state ramp)

_BUILT = {}


def _build_program(n_repeat=1):
    """Build + compile the per-core Bass program (same program on all cores).

    n_repeat > 1 unrolls the full per-inference body inside one NEFF
    (constants loaded once) -- only used for slope-timing experiments.
    """
    nc = bacc.Bacc("TRN2", target_bir_lowering=False, debug=False)

    x_d = nc.dram_tensor("x", (C, NS, 1024), FP8, kind="ExternalInput")
    w_d = [
        nc.dram_tensor(f"w{i+1}", (C, CFG[i][4] * 2 * C), FP8, kind="ExternalInput")
        for i in range(4)
    ]
    bp_d = nc.dram_tensor("bp", (C, 4), F32, kind="ExternalInput")
    cr_d = nc.dram_tensor("cr", (C, NPAIR * 2 * KCENT), FP8, kind="ExternalInput")
    cnb_d = nc.dram_tensor("cnb", (C, KCENT), F32, kind="ExternalInput")
    id_d = nc.dram_tensor("ident", (NS, NS), F32, kind="ExternalInput")
    q_d = nc.dram_tensor("q", (NS, KCENT), F32, kind="ExternalOutput")

    with tile.TileContext(nc) as tc:
        with (
            tc.tile_pool(name="consts", bufs=1) as cpool,
            tc.tile_pool(name="xp", bufs=16) as xpool,
            tc.tile_pool(name="hp", bufs=1) as hpool,
            tc.tile_pool(name="small", bufs=1) as mpool,
            tc.tile_pool(name="psA", bufs=6, space="PSUM") as psA,
            tc.tile_pool(name="psD", bufs=1, space="PSUM") as psD,
            tc.tile_pool(name="psZ", bufs=1, space="PSUM") as psZ,
        ):
            wt = [
                cpool.tile([C, CFG[i][4] * 2 * C], FP8, tag=f"w{i}", name=f"wt{i}")
                for i in range(4)
            ]
            bp = cpool.tile([C, 4], F32, tag="bp")
            cr = cpool.tile([C, NPAIR * 2 * KCENT], FP8, tag="cr")
            cnb = cpool.tile([C, KCENT], F32, tag="cnb")
            ident = cpool.tile([NS, NS], F32, tag="ident")
            onescol = cpool.tile([C, NS], F32, tag="onescol")

            for _rep in range(n_repeat):
                _body_once(nc, x_d, q_d, w_d, bp_d, cr_d, cnb_d, id_d, wt, bp,
                           cr, cnb, ident, onescol, xpool, hpool, mpool, psA,
                           psD, psZ, load_consts=(_rep == 0))

    nc.compile()
    return nc


def _body_once(nc, x_d, q_d, w_d, bp_d, cr_d, cnb_d, id_d, wt, bp, cr, cnb,
               ident, onescol, xpool, hpool, mpool, psA, psD, psZ,
               load_consts=True):
    # ---- DMA rings: x chunks stream on the SP ring; all constants ride the
    # DVE HWDGE ring (DVE is idle early) so the ACT SEQ stays free for
    # evictions and w1 races x chunk 0 ----
    if load_consts:
        wsrc = mpool.tile([1, 128], FP8, tag="warm", name="warm")
        nc.gpsimd.memset(wsrc[:], 0.0)
        nc.scalar.dma_start(wt[0][:], w_d[0].ap())
        nc.scalar.dma_start(bp[:], bp_d.ap())
    xch = []
    for g in range(16):
        t = xpool.tile([C, 2 * 1024], FP8, tag="x", name=f"xch{g}")
        src = x_d.ap()[:, 2 * g : 2 * g + 2, :].rearrange("p a b -> p (a b)")
        nc.sync.dma_start(t[:], src)
        xch.append(t)

    # ---- activation tiles ----
    h1 = hpool.tile([C, NS * 505], FP8, tag="h1")
    h2 = hpool.tile([C, NS * 248], FP8, tag="h2")  # 248-wide rows (pad col)
    h3 = hpool.tile([C, NS * 121], FP8, tag="h3")
    zt = hpool.tile([C, LPAD * NS], FP8, tag="zt")  # position-major

    # zero the slack the pad taps / pad position read (avoid fp8 NaN * 0)
    h2v = h2[:].rearrange("p (n l) -> p n l", l=248)
    nc.gpsimd.memset(h2v[:, :, 247:248], 0.0)
    nc.gpsimd.memset(zt[:, LFIN * NS : LPAD * NS], 0.0)
    if load_consts:
        nc.gpsimd.memset(onescol[:], 1.0)
        for i in range(1, 4):
            nc.gpsimd.dma_start(wt[i][:], w_d[i].ap())
        nc.gpsimd.dma_start(cr[:], cr_d.ap())
        nc.gpsimd.dma_start(cnb[:], cnb_d.ap())
        nc.gpsimd.dma_start(ident[:], id_d.ap())

    # ---- PE pre-warm: keep PE busy through the DMA lead-in so conv work is
    # billed at the ramped clock (PE idle resets the p-state window) ----
    if load_consts:
        wps = psA.tile([C, 128], F32, tag="ps", name="warmps")
        for _w in range(N_WARMUP):
            nc.tensor.matmul(
                wps[:], wsrc[:], wsrc[:], start=(_w == 0), stop=(_w == N_WARMUP - 1)
            )

    # d2 PSUM group opens with one fp32 rank-1 matmul depositing 1+||c_j||^2
    # (ones column x cnb row) -- removes a DVE add from the tail
    d_ps = psD.tile([NS, KCENT], F32, tag="d")
    nc.tensor.matmul(d_ps[:], onescol[:], cnb[:], start=True, stop=False,
                     skip_group_check=True)

    # ---- conv stack: fp8 DoubleRow tap-pair matmuls ----
    h_tiles = [None, h1, h2, h3]
    for li, (K, lin, lout, G, npr) in enumerate(CFG):
        wv = wt[li][:].rearrange("p (pp two m) -> p pp two m", two=2, m=C)
        if li > 0:
            hsrc = h_tiles[li][:].rearrange("p (n l) -> p n l", l=lin)
        for g0 in range(0, NS, G):
            ps = psA.tile([C, G * lout], F32, tag="ps")
            # DoubleRow requires a strictly 3D moving AP, so each sample gets
            # its own matmul into its PSUM column slice; weight-pair-outer
            # order reuses the stationary operand across the G samples.
            for p in range(npr):
                for i in range(G):
                    n = g0 + i
                    if li == 0:
                        x3 = xch[n // 2][:].rearrange("p (a l) -> p a l", a=2)
                        rhs = x3[:, n % 2 : n % 2 + 1, 2 * p : 2 * p + 2 * lout]
                        rhs = rhs.rearrange("p a (l two) -> p two (a l)", two=2)
                    else:
                        rhs = hsrc[:, n : n + 1, 2 * p : 2 * p + 2 * lout]
                        rhs = rhs.rearrange("p g (l two) -> p two (g l)", two=2)
                    nc.tensor.matmul(
                        ps[:, i * lout : (i + 1) * lout], wv[:, p], rhs,
                        start=(p == 0 and i == 0),
                        stop=(p == npr - 1 and i == G - 1),
                        perf_mode=PM.DoubleRow, skip_group_check=True,
                    )
            bias = bp[:, li : li + 1]
            if li < 3:
                dsl = h_tiles[li + 1][:, g0 * lout : (g0 + G) * lout]
                if li == 1:  # h2 has 248-wide rows
                    dsl = h2v[:, g0 : g0 + G, 0:247]
                nc.scalar.activation(
                    dsl, ps[:], AF.Prelu, bias=bias, scale=1.0, alpha=0.1
                )
            else:
                # conv4 eviction alternates ACT/DVE so the last-group lag
                # before the distance matmuls is halved
                zv = zt[:].rearrange("p (l n) -> p n l", n=NS)
                dsl = zv[:, g0 : g0 + G, 0:LFIN]
                if (g0 // G) % 2 == 0:
                    nc.scalar.activation(dsl, ps[:], AF.Identity, bias=bias,
                                         scale=1.0)
                else:
                    nc.vector.tensor_scalar_add(dsl, ps[:], bias)

    # ---- distance + ||z||^2: position-pair DR matmuls ----
    zpv = zt[:].rearrange("p (pp two n) -> p pp two n", two=2, n=NS)
    crv = cr[:].rearrange("p (pp two j) -> p pp two j", two=2, j=KCENT)
    g_ps = psZ.tile([NS, NS], F32, tag="g")
    for pp in range(NPAIR):
        nc.tensor.matmul(d_ps[:], zpv[:, pp], crv[:, pp], start=False,
                         stop=(pp == NPAIR - 1), perf_mode=PM.DoubleRow,
                         skip_group_check=True)
        nc.tensor.matmul(g_ps[:], zpv[:, pp], zpv[:, pp], start=(pp == 0),
                         stop=(pp == NPAIR - 1), perf_mode=PM.DoubleRow)

    # ---- q = normalize(1/(1+d2)) ----
    gd = mpool.tile([NS, NS], F32, tag="gd")
    zn1 = mpool.tile([NS, 1], F32, tag="zn1")
    nc.vector.tensor_tensor_reduce(gd[:], g_ps[:], ident[:], 1.0, 0.0,
                                   op0=OP.mult, op1=OP.add, accum_out=zn1[:])
    t1 = mpool.tile([NS, KCENT], F32, tag="t1")
    nc.vector.tensor_scalar_add(t1[:], d_ps[:], zn1[:])
    qn = mpool.tile([NS, KCENT], F32, tag="qn")
    nc.vector.reciprocal(qn[:], t1[:])
    rs = mpool.tile([NS, 1], F32, tag="rs")
    nc.vector.tensor_reduce(rs[:], qn[:], axis=mybir.AxisListType.X, op=OP.add)
    rr = mpool.tile([NS, 1], F32, tag="rr")
    nc.vector.reciprocal(rr[:], rs[:])
    nc.vector.tensor_scalar_mul(qn[:], qn[:], rr[:])
    nc.sync.dma_start(q_d.ap(), qn[:])


def _get_program(n_repeat=1):
    if n_repeat not in _BUILT:
        _BUILT[n_repeat] = _build_program(n_repeat)
    return _BUILT[n_repeat]


def _prep_inputs(x, w1, b1, w2, b2, w3, b3, w4, b4, centers):
    """Host-side prep: fp8 casts, tap-pair weight layout, per-core sharding."""
    ws = [w1, w2, w3, w4]
    bs = [b1, b2, b3, b4]

    const_map = {}
    for i, w in enumerate(ws):
        K, npr = CFG[i][0], CFG[i][4]
        wf = np.asarray(w, np.float32)  # (O, I, K)
        wp = np.zeros((C, npr, 2, C), np.float32)
        for p in range(npr):
            for j in range(2):
                k = 2 * p + j
                if k < K:
                    wp[:, p, j, :] = wf[:, :, k].T  # [i, o]
        const_map[f"w{i+1}"] = wp.reshape(C, npr * 2 * C).astype(NP8)

    bp = np.zeros((C, 4), np.float32)
    for i, b in enumerate(bs):
        bp[:, i] = np.asarray(b, np.float32)
    const_map["bp"] = bp

    cent = np.asarray(centers, np.float32)  # (64, 7552)
    c3 = cent.reshape(KCENT, C, LFIN)
    crp = np.zeros((C, NPAIR, 2, KCENT), np.float32)
    for pp in range(NPAIR):
        for j in range(2):
            l = 2 * pp + j
            if l < LFIN:
                crp[:, pp, j, :] = -2.0 * c3[:, :, l].T  # [c, t]
    const_map["cr"] = crp.reshape(C, NPAIR * 2 * KCENT).astype(NP8)

    cn = 1.0 + (cent.astype(np.float64) ** 2).sum(axis=1)  # (64,)
    # deposited into PSUM as ones(C,NS).T @ cnb: each of the C channels
    # contributes cn_j/C, summing to 1 + ||c_j||^2 (C is a power of 2)
    const_map["cnb"] = np.broadcast_to(
        (cn / C).astype(np.float32)[None, :], (C, KCENT)
    ).copy()
    const_map["ident"] = np.eye(NS, dtype=np.float32)

    xf = np.asarray(x, np.float32)
    in_maps = []
    for c in range(N_CORES):
        shard = xf[c * NS : (c + 1) * NS]  # (32, 128, 1024)
        xc = np.ascontiguousarray(shard.transpose(1, 0, 2)).astype(NP8)
        in_maps.append({"x": xc, **const_map})
    return in_maps


def _ensure_devices():
    """Absorb wedged-device attach faults with a tiny op before the real run.

    A previous process can leave a NeuronCore wedged
    (NRT_EXEC_UNIT_UNRECOVERABLE); the first attach after a wedge fails and
    triggers a reset that completes within ~60 s.
    """
    import time

    import jax
    import jax.numpy as jnp

    for attempt in range(3):
        try:
            outs = [jax.device_put(jnp.zeros((8,)), d) + 1.0 for d in jax.devices()]
            jax.block_until_ready(outs)
            return
        except Exception:  # noqa: BLE001 - device fault; wait out the reset
            if attempt == 2:
                raise
            time.sleep(60)


def run(trace=False, **inputs):
    """Run the kernel; returns (q_full, BassKernelResults).

    Retries on device-unrecoverable faults (see _ensure_devices).
    """
    import time

    _ensure_devices()
    nc = _get_program()
    in_maps = _prep_inputs(**inputs)
    last_err = None
    for attempt in range(3):
        try:
            res = bass_utils.run_bass_kernel_spmd(
                nc, in_maps, core_ids=list(range(N_CORES)), trace=trace
            )
            break
        except Exception as e:  # noqa: BLE001 - device fault, wait + retry
            last_err = e
            msg = str(e)
            retryable = any(s in msg for s in ("UNAVAILABLE", "INTERNAL")) or (
                "unrecoverable" in msg.lower()
            )
            if not retryable:
                raise
            time.sleep(60)
    else:
        raise last_err
    q = np.concatenate([res.results[c]["q"] for c in range(N_CORES)], axis=0)
    return np.ascontiguousarray(q.astype(np.float32)), res


def kernel(**inputs) -> np.ndarray:
    q, _ = run(trace=False, **inputs)
    return q


# revision 20
# speedup vs baseline: 1.1954x; 1.1954x over previous
"""Trainium2 Bass kernel for nn_DEC_26139170963600 (vq_codebook).

Reference computation:
  4x strided conv1d (stride 2, VALID) with LeakyReLU(0.1) between layers,
  flatten -> soft VQ assignment over 64 centers:
      d2 = ||z||^2 + ||c||^2 - 2 z.c
      q  = (1/(1+d2)) row-normalized            (alpha=1 -> exponent is 1)

Sharding: data-parallel over batch N=256 across 8 cores (32 samples/core).
Weights / centers replicated. No cross-device communication.

Per-core kernel design (all-fp8e4 conv path, ~2.3e-3 max rel err vs fp32):
  - Convs run as fp8 DoubleRow matmuls: each instruction contracts TWO taps
    (lhsT (128,2,128) pair of tap-weight matrices, rhs (128,2,G,Lout) pair of
    strided input slices) at 0.5 PE cycles per output column -- 4x the bf16
    rate per MAC.  Odd tap counts are padded with a zero tap (conv1 15->16,
    conv3 7->8); the pad tap's reads stay finite (slack columns memset to 0).
  - PSUM eviction is a single ACT op: Prelu(y + b) with alpha=0.1 (HW Prelu is
    exact; HW Lrelu is broken - returns 0.01x on negatives).  Outputs written
    directly as fp8 for the next layer.  conv4 evicts Identity+bias into z.
  - z is stored position-major (l*32 + n) so the distance matmul pairs two
    positions per DR instruction with 16-aligned pair strides:
      d2 cross term: lhsT = z-pair (128,2,32), rhs = cr-pair (128,2,64)
      ||z||^2:       same lhsT vs itself -> (32,32) Gram, diag extracted with
                     a (32,32) identity mult + row-reduce on DVE.
    1 + ||c||^2 comes in as a host-precomputed (32,64) fp32 tile (exact).
  - q = reciprocal(1+d2) row-normalized on DVE, DMA out as fp32.
  - PE pre-warm: dummy matmuls bridge the DMA lead-in so the cost of real conv
    work is billed at the full 2.4 GHz p-state (PE clock ramps over ~3us).

fp16 matmuls hard-fault trn2 here (NRT_EXEC_UNIT_UNRECOVERABLE) - do not use.
"""

import os
import sys

import numpy as np
import ml_dtypes

for _p in ("/opt/trn_rl_repo",):
    if _p not in sys.path and os.path.isdir(_p):
        sys.path.insert(0, _p)

import concourse.bacc as bacc  # noqa: E402
import concourse.mybir as mybir  # noqa: E402
import concourse.tile as tile  # noqa: E402
from concourse import bass_utils  # noqa: E402

FP8 = mybir.dt.float8e4
F32 = mybir.dt.float32
AF = mybir.ActivationFunctionType
OP = mybir.AluOpType
PM = mybir.MatmulPerfMode
NP8 = ml_dtypes.float8_e4m3  # maps to mybir float8e4 (max +-240)

N_CORES = 8
NS = 32          # samples per core
C = 128          # channels
KCENT = 64       # number of centers
LFIN = 59        # final length
LPAD = 60        # padded (even) final length for position-pair matmuls
NPAIR = LPAD // 2

# (K taps, L_in row stride, L_out, G samples per matmul, tap pairs)
CFG = [
    (15, 1024, 505, 1, 8),
    (12, 505, 247, 2, 6),
    (7, 248, 121, 4, 4),   # h2 rows padded 247->248 (pad-tap read slack)
    (4, 121, 59, 8, 2),
]
N_WARMUP = 24  # dummy PE matmuls bridging the DMA lead-in (p-state ramp)

_BUILT = {}


def _build_program(n_repeat=1):
    """Build + compile the per-core Bass program (same program on all cores).

    n_repeat > 1 unrolls the full per-inference body inside one NEFF
    (constants loaded once) -- only used for slope-timing experiments.
    """
    nc = bacc.Bacc("TRN2", target_bir_lowering=False, debug=False)

    x_d = nc.dram_tensor("x", (C, NS, 1024), FP8, kind="ExternalInput")
    w_d = [
        nc.dram_tensor(f"w{i+1}", (C, CFG[i][4] * 2 * C), FP8, kind="ExternalInput")
        for i in range(4)
    ]
    bp_d = nc.dram_tensor("bp", (C, 4), F32, kind="ExternalInput")
    cr_d = nc.dram_tensor("cr", (C, NPAIR * 2 * KCENT), FP8, kind="ExternalInput")
    cnb_d = nc.dram_tensor("cnb", (C, KCENT), F32, kind="ExternalInput")
    id_d = nc.dram_tensor("ident", (NS, NS), F32, kind="ExternalInput")
    q_d = nc.dram_tensor("q", (NS, KCENT), F32, kind="ExternalOutput")

    with tile.TileContext(nc) as tc:
        with (
            tc.tile_pool(name="consts", bufs=1) as cpool,
            tc.tile_pool(name="xp", bufs=16) as xpool,
            tc.tile_pool(name="hp", bufs=1) as hpool,
            tc.tile_pool(name="small", bufs=1) as mpool,
            tc.tile_pool(name="psA", bufs=6, space="PSUM") as psA,
            tc.tile_pool(name="psD", bufs=1, space="PSUM") as psD,
            tc.tile_pool(name="psZ", bufs=1, space="PSUM") as psZ,
        ):
            wt = [
                cpool.tile([C, CFG[i][4] * 2 * C], FP8, tag=f"w{i}", name=f"wt{i}")
                for i in range(4)
            ]
            bp = cpool.tile([C, 4], F32, tag="bp")
            cr = cpool.tile([C, NPAIR * 2 * KCENT], FP8, tag="cr")
            cnb = cpool.tile([C, KCENT], F32, tag="cnb")
            ident = cpool.tile([NS, NS], F32, tag="ident")
            onescol = cpool.tile([C, NS], F32, tag="onescol")

            for _rep in range(n_repeat):
                _body_once(nc, x_d, q_d, w_d, bp_d, cr_d, cnb_d, id_d, wt, bp,
                           cr, cnb, ident, onescol, xpool, hpool, mpool, psA,
                           psD, psZ, load_consts=(_rep == 0))

    nc.compile()
    return nc


def _body_once(nc, x_d, q_d, w_d, bp_d, cr_d, cnb_d, id_d, wt, bp, cr, cnb,
               ident, onescol, xpool, hpool, mpool, psA, psD, psZ,
               load_consts=True):
    # ---- DMA rings: x chunks stream on the SP ring; all constants ride the
    # DVE HWDGE ring (DVE is idle early) so the ACT SEQ stays free for
    # evictions and w1 races x chunk 0 ----
    if load_consts:
        wsrc = mpool.tile([1, 128], FP8, tag="warm", name="warm")
        nc.gpsimd.memset(wsrc[:], 0.0)
        nc.sync.dma_start(wt[0][:], w_d[0].ap())  # w1 leads the SP ring
    xch = []
    for g in range(16):
        t = xpool.tile([C, 2 * 1024], FP8, tag="x", name=f"xch{g}")
        src = x_d.ap()[:, 2 * g : 2 * g + 2, :].rearrange("p a b -> p (a b)")
        # chunks 0/2/4 ride the otherwise-idle ACT ring so the two HWDGE
        # rings pipeline the conv1 lead-in in parallel
        if g in (0, 2, 4):
            nc.scalar.dma_start(t[:], src)
            if g == 0 and load_consts:
                nc.scalar.dma_start(bp[:], bp_d.ap())
        else:
            nc.sync.dma_start(t[:], src)
        xch.append(t)

    # ---- activation tiles ----
    h1 = hpool.tile([C, NS * 505], FP8, tag="h1")
    h2 = hpool.tile([C, NS * 248], FP8, tag="h2")  # 248-wide rows (pad col)
    h3 = hpool.tile([C, NS * 121], FP8, tag="h3")
    zt = hpool.tile([C, LPAD * NS], FP8, tag="zt")  # position-major

    # zero the slack the pad taps / pad position read (avoid fp8 NaN * 0)
    h2v = h2[:].rearrange("p (n l) -> p n l", l=248)
    nc.gpsimd.memset(h2v[:, :, 247:248], 0.0)
    nc.gpsimd.memset(zt[:, LFIN * NS : LPAD * NS], 0.0)
    if load_consts:
        nc.gpsimd.memset(onescol[:], 1.0)
        # late-needed consts ride the SP ring behind the x chunks (the SP SEQ
        # clears them by ~15us; first consumer is conv2 at ~30us)
        for i in range(1, 4):
            nc.sync.dma_start(wt[i][:], w_d[i].ap())
        nc.sync.dma_start(cr[:], cr_d.ap())
        nc.sync.dma_start(cnb[:], cnb_d.ap())
        nc.sync.dma_start(ident[:], id_d.ap())

    # ---- PE pre-warm: keep PE busy through the DMA lead-in so conv work is
    # billed at the ramped clock (PE idle resets the p-state window) ----
    if load_consts:
        wps = psA.tile([C, 128], F32, tag="ps", name="warmps")
        for _w in range(N_WARMUP):
            nc.tensor.matmul(
                wps[:], wsrc[:], wsrc[:], start=(_w == 0), stop=(_w == N_WARMUP - 1)
            )

    # ---- conv stack: fp8 DoubleRow tap-pair matmuls ----
    h_tiles = [None, h1, h2, h3]
    for li, (K, lin, lout, G, npr) in enumerate(CFG):
        wv = wt[li][:].rearrange("p (pp two m) -> p pp two m", two=2, m=C)
        if li > 0:
            hsrc = h_tiles[li][:].rearrange("p (n l) -> p n l", l=lin)
        for g0 in range(0, NS, G):
            ps = psA.tile([C, G * lout], F32, tag="ps")
            # DoubleRow requires a strictly 3D moving AP, so each sample gets
            # its own matmul into its PSUM column slice; weight-pair-outer
            # order reuses the stationary operand across the G samples.
            for p in range(npr):
                for i in range(G):
                    n = g0 + i
                    if li == 0:
                        x3 = xch[n // 2][:].rearrange("p (a l) -> p a l", a=2)
                        rhs = x3[:, n % 2 : n % 2 + 1, 2 * p : 2 * p + 2 * lout]
                        rhs = rhs.rearrange("p a (l two) -> p two (a l)", two=2)
                    else:
                        rhs = hsrc[:, n : n + 1, 2 * p : 2 * p + 2 * lout]
                        rhs = rhs.rearrange("p g (l two) -> p two (g l)", two=2)
                    nc.tensor.matmul(
                        ps[:, i * lout : (i + 1) * lout], wv[:, p], rhs,
                        start=(p == 0 and i == 0),
                        stop=(p == npr - 1 and i == G - 1),
                        perf_mode=PM.DoubleRow, skip_group_check=True,
                    )
            bias = bp[:, li : li + 1]
            if li < 3:
                dsl = h_tiles[li + 1][:, g0 * lout : (g0 + G) * lout]
                if li == 1:  # h2 has 248-wide rows
                    dsl = h2v[:, g0 : g0 + G, 0:247]
                nc.scalar.activation(
                    dsl, ps[:], AF.Prelu, bias=bias, scale=1.0, alpha=0.1
                )
            else:
                # conv4 eviction alternates ACT/DVE so the last-group lag
                # before the distance matmuls is halved
                zv = zt[:].rearrange("p (l n) -> p n l", n=NS)
                dsl = zv[:, g0 : g0 + G, 0:LFIN]
                if (g0 // G) % 2 == 0:
                    nc.scalar.activation(dsl, ps[:], AF.Identity, bias=bias,
                                         scale=1.0)
                else:
                    nc.vector.tensor_scalar_add(dsl, ps[:], bias)

    # ---- distance + ||z||^2: position-pair DR matmuls ----
    # d2 PSUM group opens with one fp32 rank-1 matmul depositing 1+||c_j||^2
    # (ones column x cnb row) -- removes a DVE add from the tail
    d_ps = psD.tile([NS, KCENT], F32, tag="d")
    nc.tensor.matmul(d_ps[:], onescol[:], cnb[:], start=True, stop=False,
                     skip_group_check=True)
    zpv = zt[:].rearrange("p (pp two n) -> p pp two n", two=2, n=NS)
    crv = cr[:].rearrange("p (pp two j) -> p pp two j", two=2, j=KCENT)
    g_ps = psZ.tile([NS, NS], F32, tag="g")
    for pp in range(NPAIR):
        nc.tensor.matmul(d_ps[:], zpv[:, pp], crv[:, pp], start=False,
                         stop=(pp == NPAIR - 1), perf_mode=PM.DoubleRow,
                         skip_group_check=True)
        nc.tensor.matmul(g_ps[:], zpv[:, pp], zpv[:, pp], start=(pp == 0),
                         stop=(pp == NPAIR - 1), perf_mode=PM.DoubleRow)

    # ---- q = normalize(1/(1+d2)) ----
    gd = mpool.tile([NS, NS], F32, tag="gd")
    zn1 = mpool.tile([NS, 1], F32, tag="zn1")
    nc.vector.tensor_tensor_reduce(gd[:], g_ps[:], ident[:], 1.0, 0.0,
                                   op0=OP.mult, op1=OP.add, accum_out=zn1[:])
    t1 = mpool.tile([NS, KCENT], F32, tag="t1")
    nc.vector.tensor_scalar_add(t1[:], d_ps[:], zn1[:])
    qn = mpool.tile([NS, KCENT], F32, tag="qn")
    nc.vector.reciprocal(qn[:], t1[:])
    rs = mpool.tile([NS, 1], F32, tag="rs")
    nc.vector.tensor_reduce(rs[:], qn[:], axis=mybir.AxisListType.X, op=OP.add)
    rr = mpool.tile([NS, 1], F32, tag="rr")
    nc.vector.reciprocal(rr[:], rs[:])
    nc.vector.tensor_scalar_mul(qn[:], qn[:], rr[:])
    nc.sync.dma_start(q_d.ap(), qn[:])


def _get_program(n_repeat=1):
    if n_repeat not in _BUILT:
        _BUILT[n_repeat] = _build_program(n_repeat)
    return _BUILT[n_repeat]


def _prep_inputs(x, w1, b1, w2, b2, w3, b3, w4, b4, centers):
    """Host-side prep: fp8 casts, tap-pair weight layout, per-core sharding."""
    ws = [w1, w2, w3, w4]
    bs = [b1, b2, b3, b4]

    const_map = {}
    for i, w in enumerate(ws):
        K, npr = CFG[i][0], CFG[i][4]
        wf = np.asarray(w, np.float32)  # (O, I, K)
        wp = np.zeros((C, npr, 2, C), np.float32)
        for p in range(npr):
            for j in range(2):
                k = 2 * p + j
                if k < K:
                    wp[:, p, j, :] = wf[:, :, k].T  # [i, o]
        const_map[f"w{i+1}"] = wp.reshape(C, npr * 2 * C).astype(NP8)

    bp = np.zeros((C, 4), np.float32)
    for i, b in enumerate(bs):
        bp[:, i] = np.asarray(b, np.float32)
    const_map["bp"] = bp

    cent = np.asarray(centers, np.float32)  # (64, 7552)
    c3 = cent.reshape(KCENT, C, LFIN)
    crp = np.zeros((C, NPAIR, 2, KCENT), np.float32)
    for pp in range(NPAIR):
        for j in range(2):
            l = 2 * pp + j
            if l < LFIN:
                crp[:, pp, j, :] = -2.0 * c3[:, :, l].T  # [c, t]
    const_map["cr"] = crp.reshape(C, NPAIR * 2 * KCENT).astype(NP8)

    cn = 1.0 + (cent.astype(np.float64) ** 2).sum(axis=1)  # (64,)
    # deposited into PSUM as ones(C,NS).T @ cnb: each of the C channels
    # contributes cn_j/C, summing to 1 + ||c_j||^2 (C is a power of 2)
    const_map["cnb"] = np.broadcast_to(
        (cn / C).astype(np.float32)[None, :], (C, KCENT)
    ).copy()
    const_map["ident"] = np.eye(NS, dtype=np.float32)

    xf = np.asarray(x, np.float32)
    in_maps = []
    for c in range(N_CORES):
        shard = xf[c * NS : (c + 1) * NS]  # (32, 128, 1024)
        xc = np.ascontiguousarray(shard.transpose(1, 0, 2)).astype(NP8)
        in_maps.append({"x": xc, **const_map})
    return in_maps


def _ensure_devices():
    """Absorb wedged-device attach faults with a tiny op before the real run.

    A previous process can leave a NeuronCore wedged
    (NRT_EXEC_UNIT_UNRECOVERABLE); the first attach after a wedge fails and
    triggers a reset that completes within ~60 s.
    """
    import time

    import jax
    import jax.numpy as jnp

    for attempt in range(3):
        try:
            outs = [jax.device_put(jnp.zeros((8,)), d) + 1.0 for d in jax.devices()]
            jax.block_until_ready(outs)
            return
        except Exception:  # noqa: BLE001 - device fault; wait out the reset
            if attempt == 2:
                raise
            time.sleep(60)


def run(trace=False, **inputs):
    """Run the kernel; returns (q_full, BassKernelResults).

    Retries on device-unrecoverable faults (see _ensure_devices).
    """
    import time

    _ensure_devices()
    nc = _get_program()
    in_maps = _prep_inputs(**inputs)
    last_err = None
    for attempt in range(3):
        try:
            res = bass_utils.run_bass_kernel_spmd(
                nc, in_maps, core_ids=list(range(N_CORES)), trace=trace
            )
            break
        except Exception as e:  # noqa: BLE001 - device fault, wait + retry
            last_err = e
            msg = str(e)
            retryable = any(s in msg for s in ("UNAVAILABLE", "INTERNAL")) or (
                "unrecoverable" in msg.lower()
            )
            if not retryable:
                raise
            time.sleep(60)
    else:
        raise last_err
    q = np.concatenate([res.results[c]["q"] for c in range(N_CORES)], axis=0)
    return np.ascontiguousarray(q.astype(np.float32)), res


def kernel(**inputs) -> np.ndarray:
    q, _ = run(trace=False, **inputs)
    return q
